# revision 33
# baseline (speedup 1.0000x reference)
"""Trainium2 Bass kernel for nn_CADenseAdd (context-adaptive low-rank dense + ReLU).

Reference math (per batch row b):
    s_b   = S + context_b @ W                  # [RANK]
    out_b = relu((x_b @ U) * s_b @ V.T + bias) # [UNITS]

Sharding: data-parallel over batch B=2048 across 8 cores (256 rows/core);
U/S/V/W replicated.  All matmuls are done "transposed" so the contraction
dim always lands on SBUF partitions with zero on-device transposes:

    sT  = W_aug^T @ ctxT_aug      [RANK,  BS]  (S folded in on the host)
    xuT = U^T @ xT                [RANK,  BS]
    tT  = xuT * sT  (cast fp16)   [RANK,  BS]
    outT[um] = Vt[um] @ tT        [UNITS, BS]  (+bias, ReLU on eviction)

Measured pipeline facts (ntff traces, 2026-08-08): the two HWDGE rings
share one ~400 GB/s HBM read pipe (splitting inputs across rings only
de-paces the stream), small dmas at the stream head crater ring
throughput, the final ~0.3MB of any stream trickles at <50 GB/s, and
store queues cap around 120-190 GB/s each.  Hence: ALL inputs ride the
sync ring in need-order with >=0.25MB chunks (u/x head -> ctx -> W in
two pieces -> x/U bulk -> tapered tail -> Vt), mm1 is woven between
mm2's head kn-steps as stream filler, mm2's last two kn-steps run
rm-major so DVE's tT multiplies overlap the PE tail, and each eviction
group stages into ONE combined DVE/ACT tile stored with a single
contiguous DMA, alternating between the scalar HWDGE and gpsimd SWDGE
queues (the last, smallest group trails the final matmul by ~1us).
Engine-queue pipelining means same-engine RAW/WAW still needs semaphore
self-waits, and walrus encodes at most ONE wait per instruction (matmul
weights-waits ride the LDWEIGHTS) -- the fence/observer instructions
below exist to satisfy exactly that.
"""

import re

import numpy as np

import bass_rust
import concourse.bass as bass
import concourse.tile as tile
from concourse import mybir
from concourse.bass_utils import run_bass_kernel_spmd
from concourse.vector_clock import ScopedClock


def _split_drain_and_barrier(self, tick_clock, wait_clock):
    """Replacement for TileContext._drain_and_barrier.

    The walrus build in this toolchain cannot encode more than one sync
    wait per instruction ("Too many sync wait commands"), and Tile's final
    drain carries one wait per active proc (~12 here).  Emit those waits as
    a chain of single-wait SP nops instead, then a bare drain: the SP queue
    executes in order, so the drain still happens after every proc's final
    tick.
    """
    ticks = [int(x) for x in re.findall(r"\d+", repr(tick_clock.global_clock))]
    for proc, tick in enumerate(ticks):
        if tick > 0:
            nop_inst = self.nc.sync.nop(nofuse=True)
            sub = bass_rust.VectorClock()
            sub.require_at_least(proc, tick)
            wait_clock.add_sem_waits(nop_inst.ins, ScopedClock({None: sub}))
    self.nc.sync.drain()
    self.nc.all_engine_barrier()
    popped = self.nc._tile_sem_poison_stack.pop()
    assert popped is self._sem_poison
    self.nc.clear_and_free_semaphores(list(self.sems.allocated().values()))
    self.nc.all_engine_barrier()


tile.TileContext._drain_and_barrier = _split_drain_and_barrier

# Problem shape (hardcoded per contract)
M = 8  # cores
B, N, C = 2048, 4096, 1024
UNITS, RANK = 4096, 512
BS = B // M  # 256 rows per core
P = 128
KN = N // P      # 32 contraction tiles for x @ U
KC = C // P      # 8 contraction tiles for ctx @ W
RM = RANK // P   # 4 tiles of RANK
UM = UNITS // P  # 32 tiles of UNITS

F16 = mybir.dt.float16
F32 = mybir.dt.float32


N_WARM_MM = 10  # ~2.1us of cold-rate dummies: covers the DMA stream head

# ALL inputs ride the sync ring in need-order (the two HWDGE rings share
# one ~400 GB/s HBM pipe, so splitting inputs across rings adds nothing;
# SMALL dmas at the stream head crater ring throughput, so every chunk
# is >=0.25MB and the first is 0.5MB).  Order: a u/x head feeds mm2's
# first kn-steps, ctx and W (split in two for mm1 sem granularity) flow
# next while the PE chews mm1, then the x/U bulk with a tapered tail
# (small last chunks shrink the after-stream compute tail), then Vt.
SYNC_SCHED = [
    ("u", 0, 8), ("x", 0, 8), ("ctx", 0, 9), ("w", 0, 9),
    ("x", 8, 16), ("u", 8, 16), ("x", 16, 24), ("u", 16, 24),
    ("x", 24, 32), ("u", 24, 32),
]
V_CHUNKS = [(0, 8), (8, 16), (16, 24), (24, 32)]

# PE phase-1 order tracks the stream: mm2's first kn-steps, then mm1
# (ctx/W arrive mid-stream), then the rest of mm2.  PE idle at the head
# is harmless (the PE catches the stream mid-phase); what matters is the
# stream's END time, so every head dma is as big as possible.
PHASE1_OPS = (
    [("kn", j) for j in range(8)]
    + [("kc", i) for i in range(KC + 1)]
    + [("kn", j) for j in range(8, KN - 4)]
)
# Last four kn steps run rm-major so ps_xu[rm] completes staggered: DVE's
# tT multiplies overlap the PE's remaining tail matmuls instead of
# serializing at the phase boundary (tT3 lands ~0.4us after the last
# tail matmul, so the boundary ldweights barely waits).
PHASE1_TAIL_KNS = [KN - 4, KN - 3, KN - 2, KN - 1]


def build_program(zero_bias: bool = True) -> bass.Bass:
    """Build the per-core SPMD program.

    Wait-encoding constraint: this walrus build cannot encode >1 sem-wait
    on DVE/ACT tensor instructions, while matmuls can encode 2.  Every
    DVE/ACT instruction below carries <=1 wait (PSUM source tick, or a DMA
    tick observed once); matmuls carry at most a DMA-lane tick plus a DVE
    tick.
    """
    nc = bass.Bass("TRN2", debug=False, enable_asserts=False, enable_partition_id=False, dynamic_dma_scratch_size=4096)

    # S is folded into mm1 on the host: ctxT/W carry an extra contraction
    # tile (ones-row / S-row), so sT = W_aug^T @ ctxT_aug exactly.
    KC1 = KC + 1
    xT_d = nc.dram_tensor("xT", [P, KN, BS], F16, kind="ExternalInput").ap()
    ctxT_d = nc.dram_tensor("ctxT", [P, KC1, BS], F16, kind="ExternalInput").ap()
    U_d = nc.dram_tensor("U", [P, KN, RANK], F16, kind="ExternalInput").ap()
    W_d = nc.dram_tensor("W", [P, KC1, RANK], F16, kind="ExternalInput").ap()
    V3_d = nc.dram_tensor("V3", [P, UM, RM, P], F16, kind="ExternalInput").ap()
    if not zero_bias:
        bias_d = nc.dram_tensor("bias", [P, UM], F32, kind="ExternalInput").ap()
    # outT3[p, e, i, b] = output tile um=2i+e, i.e. DVE-evicted (even um)
    # and ACT-evicted (odd um) halves live in separate contiguous planes.
    outT3_d = nc.dram_tensor("outT3", [P, 2, UM // 2, BS], F16, kind="ExternalOutput").ap()

    with tile.TileContext(nc) as tc:
        with (
            tc.tile_pool(name="consts", bufs=1) as cpool,
            tc.tile_pool(name="ctxp", bufs=1) as ctxpool,
            tc.tile_pool(name="wp", bufs=1) as wpool,
            tc.tile_pool(name="xp", bufs=1) as xpool,
            tc.tile_pool(name="up", bufs=1) as upool,
            tc.tile_pool(name="vp", bufs=1) as vpool,
            tc.tile_pool(name="actp", bufs=1) as actpool,
            tc.tile_pool(name="ogp", bufs=1) as ogpool,
        ):
            # PSUM: mm1 takes 4 banks (warm dummy shares via tag), mm2 the
            # other 4; both released before mm3's 6-deep eviction pipeline.
            ps_s_pool = tc.alloc_tile_pool(name="pss", bufs=4, space="PSUM")
            ps_xu_pool = tc.alloc_tile_pool(name="psxu", bufs=4, space="PSUM")

            if not zero_bias:
                b_sb = cpool.tile([P, UM], F32, name="b_sb")
                nc.scalar.dma_start(b_sb[:], bias_d[:])

            # ---- sync-ring input loads, all in need-order ----
            x_of_kn: dict = {}
            u_of_kn: dict = {}
            ctx_of_kc: dict = {}
            w_of_kc: dict = {}
            for kind, lo, hi in SYNC_SCHED:
                if kind == "x":
                    t = xpool.tile([P, hi - lo, BS], F16, name=f"x{lo}")
                    nc.sync.dma_start(t[:], xT_d[:, lo:hi, :])
                    for kn in range(lo, hi):
                        x_of_kn[kn] = t[:, kn - lo, :]
                elif kind == "u":
                    t = upool.tile([P, hi - lo, RANK], F16, name=f"u{lo}")
                    nc.sync.dma_start(t[:], U_d[:, lo:hi, :])
                    for kn in range(lo, hi):
                        u_of_kn[kn] = t[:, kn - lo, :]
                elif kind == "ctx":
                    t = ctxpool.tile([P, hi - lo, BS], F16, name="ctx_sb")
                    nc.sync.dma_start(t[:], ctxT_d[:, lo:hi, :])
                    for kc in range(lo, hi):
                        ctx_of_kc[kc] = t[:, kc - lo, :]
                else:
                    t = wpool.tile([P, hi - lo, RANK], F16, name=f"w{lo}")
                    nc.sync.dma_start(t[:], W_d[:, lo:hi, :])
                    for kc in range(lo, hi):
                        w_of_kc[kc] = t[:, kc - lo, :]
            vt_of_um: dict = {}
            for lo, hi in V_CHUNKS:
                vt = vpool.tile([P, hi - lo, RM, P], F16, name=f"v{lo}")
                nc.sync.dma_start(vt[:], V3_d[:, lo:hi, :, :])
                for um in range(lo, hi):
                    vt_of_um[um] = vt[:, um - lo, :, :]

            # ---- engine warm-up during the DMA head ----
            warm_src = cpool.tile([P, BS + P], F16, name="warm_src")
            nc.gpsimd.memset(warm_src[:], 0.0)
            act_scr = cpool.tile([P, 1], F16, name="act_scr")
            ps_warm = ps_s_pool.tile([P, BS], F32, name="ps_warm", tag="s")
            for _ in range(N_WARM_MM):
                nc.tensor.matmul(
                    ps_warm[:], lhsT=warm_src[:, BS:], rhs=warm_src[:, :BS],
                    start=True, stop=True,
                )
            # ACT loads its Relu table now (Q14, off the input rings) and
            # pre-touches bias so evictions keep <=1 wait.
            nc.scalar.activation(
                act_scr[:], warm_src[:, :1],
                mybir.ActivationFunctionType.Relu, bias=0.0,
            )
            if not zero_bias:
                dve_scr2 = cpool.tile([P, UM], F32, name="dve_scr2")
                nc.vector.tensor_copy(dve_scr2[:], b_sb[:])
                act_scr2 = cpool.tile([P, UM], F32, name="act_scr2")
                nc.scalar.copy(act_scr2[:], b_sb[:])

            # ---- phase 1: mm1 (sT) woven into mm2 (xuT) ----
            ps_s = [
                ps_s_pool.tile([P, BS], F32, name=f"ps_s{rm}", tag="s")
                for rm in range(RM)
            ]
            ps_xu = [
                ps_xu_pool.tile([P, BS], F32, name=f"ps_xu{rm}", tag="xu")
                for rm in range(RM)
            ]
            for kind, i in PHASE1_OPS:
                if kind == "kc":
                    for rm in range(RM):
                        nc.tensor.matmul(
                            ps_s[rm][:],
                            lhsT=w_of_kc[i][:, rm * P : (rm + 1) * P],
                            rhs=ctx_of_kc[i],
                            start=(i == 0),
                            stop=(i == KC1 - 1),
                        )
                else:
                    for rm in range(RM):
                        nc.tensor.matmul(
                            ps_xu[rm][:],
                            lhsT=u_of_kn[i][:, rm * P : (rm + 1) * P],
                            rhs=x_of_kn[i],
                            start=(i == 0),
                            stop=False,
                        )

            sT = [actpool.tile([P, BS], F32, name=f"sT{rm}") for rm in range(RM)]
            for rm in range(RM):
                nc.vector.tensor_copy(sT[rm][:], ps_s[rm][:])
            # DVE fence: observe sT3's tick so the tT multiplies carry only
            # their PE wait (DVE self-RAW needs a semaphore otherwise).
            dve_scr = cpool.tile([P, RM], F32, name="dve_scr")
            nc.vector.tensor_copy(dve_scr[:, :1], sT[RM - 1][:, :1])

            tT = [actpool.tile([P, BS], F16, name=f"tT{rm}") for rm in range(RM)]
            for rm in range(RM):
                for i in PHASE1_TAIL_KNS:
                    nc.tensor.matmul(
                        ps_xu[rm][:],
                        lhsT=u_of_kn[i][:, rm * P : (rm + 1) * P],
                        rhs=x_of_kn[i],
                        start=False,
                        stop=(i == KN - 1),
                    )
                nc.vector.tensor_mul(tT[rm][:], ps_xu[rm][:], sT[rm][:])

            ps_xu_pool.release()
            ps_s_pool.release()
            ps_o_pool = tc.alloc_tile_pool(name="pso", bufs=6, space="PSUM")

            # Phase-boundary fences: each engine observes the boundary ticks
            # once so mm3's instructions carry <=1 wait each.  PE observes the
            # DVE tick via a standalone ldweights, then one dummy matmul
            # absorbs the released-bank WAW tick (engine queues stall on
            # waits, so later PE instructions inherit both).
            nc.tensor.ldweights(tT[RM - 1][:, :P])
            ps_fence = ps_o_pool.tile([P, BS], F32, name="ps_fence", tag="pso")
            nc.tensor.matmul(
                ps_fence[:], lhsT=warm_src[:, BS:], rhs=warm_src[:, :BS],
                start=True, stop=True,
            )
            nc.vector.tensor_copy(dve_scr[:, 1:2], tT[RM - 1][:, :1])
            act_fence_scr = cpool.tile([P, 1], F16, name="act_fence_scr")
            nc.scalar.copy(act_fence_scr[:], tT[RM - 1][:, :1])

            # ---- mm3: outT[um] = relu(Vt[um] @ tT + bias[um]) ----
            # No phase fences: mm3's first matmuls carry the DVE tick (tT
            # data + released-bank WAW coalesce into one DVE wait) plus the
            # Vt DMA-lane tick -- within the PE's 2-wait budget.
            # Evictions alternate DVE/ACT into ONE combined staging tile per
            # group; a single contiguous DMA per group stores both halves.
            group_sizes = [8, 8, 8, 6, 2]
            assert sum(group_sizes) == UM
            um0 = 0
            for g, gs in enumerate(group_sizes):
                og = ogpool.tile([P, 2, gs // 2, BS], F16, name=f"og{g}")
                obs = cpool.tile([P, 1], F16, name=f"obs{g}")
                obs2 = cpool.tile([P, 1], F16, name=f"obs2{g}")
                for j in range(gs):
                    um = um0 + j
                    ps_o = ps_o_pool.tile([P, BS], F32, name="ps_o", tag="pso")
                    vt = vt_of_um[um]  # [P, RM, P]
                    for kr in range(RM):
                        nc.tensor.matmul(
                            ps_o[:],
                            lhsT=vt[:, kr, :],
                            rhs=tT[kr][:],
                            start=(kr == 0),
                            stop=(kr == RM - 1),
                        )
                    if zero_bias:
                        if j % 2 == 0:
                            nc.vector.tensor_scalar_max(
                                og[:, 0, j // 2, :], ps_o[:], 0.0
                            )
                        else:
                            nc.scalar.activation(
                                og[:, 1, j // 2, :], ps_o[:],
                                mybir.ActivationFunctionType.Relu, bias=0.0,
                            )
                    else:
                        if j % 2 == 0:
                            nc.vector.tensor_tensor(
                                og[:, 0, j // 2, :], ps_o[:],
                                b_sb[:, um : um + 1].to_broadcast((P, BS)),
                                mybir.AluOpType.add,
                            )
                            nc.vector.tensor_scalar_max(
                                og[:, 0, j // 2, :], og[:, 0, j // 2, :], 0.0
                            )
                        else:
                            nc.scalar.activation(
                                og[:, 1, j // 2, :], ps_o[:],
                                mybir.ActivationFunctionType.Relu,
                                bias=b_sb[:, um : um + 1],
                            )
                # Stores alternate between the scalar HWDGE ring and the
                # gpsimd SWDGE ring (a single store queue caps at ~187 GB/s,
                # about half the read rate).  The issuing engine observes the
                # last write of each og plane first (one observer copy per
                # producer proc), so the store needs only its DMA-lane wait.
                out_slice = outT3_d[:, :, um0 // 2 : (um0 + gs) // 2, :]
                if g % 2 == 0:
                    nc.scalar.copy(obs[:], og[:, 0, gs // 2 - 1, :1])
                    nc.scalar.copy(obs2[:], og[:, 1, gs // 2 - 1, :1])
                    nc.scalar.dma_start(out_slice, og[:])
                else:
                    nc.gpsimd.tensor_copy(obs[:], og[:, 0, gs // 2 - 1, :1])
                    nc.gpsimd.tensor_copy(obs2[:], og[:, 1, gs // 2 - 1, :1])
                    nc.gpsimd.dma_start(out_slice, og[:])
                um0 += gs

            ps_o_pool.release()

    return nc


def _pack_inputs(inputs, context, U, S, V, W, bias):
    """Shard + pack the full fp32 inputs into per-core [128,...] fp16 layouts.

    S is folded into the mm1 operands: ctxT gets a 9th contraction tile that
    is a ones-row (partition 0 only), W gets a matching row carrying S, so
    sT = W_aug^T @ ctxT_aug = S + W^T @ ctxT exactly.
    """
    zero_bias = not bias.any()
    x16 = inputs.astype(np.float16)
    c16 = context.astype(np.float16)
    U_pk = np.ascontiguousarray(U.astype(np.float16).reshape(KN, P, RANK).transpose(1, 0, 2))
    W_pk = np.zeros((P, KC + 1, RANK), dtype=np.float16)
    W_pk[:, :KC, :] = W.astype(np.float16).reshape(KC, P, RANK).transpose(1, 0, 2)
    W_pk[0, KC, :] = S.astype(np.float16)
    # V3[p, um, kr, c] = V[um*128 + c, kr*128 + p]
    V3_pk = np.ascontiguousarray(
        V.astype(np.float16).reshape(UM, P, RM, P).transpose(3, 0, 2, 1)
    )
    b_pk = np.ascontiguousarray(bias.astype(np.float32).reshape(UM, P).T)

    in_maps = []
    for c in range(M):
        xs = x16[c * BS : (c + 1) * BS]  # [BS, N]
        cs = c16[c * BS : (c + 1) * BS]  # [BS, C]
        xT = np.ascontiguousarray(xs.T.reshape(KN, P, BS).transpose(1, 0, 2))
        ctxT = np.zeros((P, KC + 1, BS), dtype=np.float16)
        ctxT[:, :KC, :] = cs.T.reshape(KC, P, BS).transpose(1, 0, 2)
        ctxT[0, KC, :] = 1.0
        im = {"xT": xT, "ctxT": ctxT, "U": U_pk, "W": W_pk, "V3": V3_pk}
        if not zero_bias:
            im["bias"] = b_pk
        in_maps.append(im)
    return in_maps


_PROGRAM_CACHE = {}


def _get_program(zero_bias: bool) -> bass.Bass:
    if zero_bias not in _PROGRAM_CACHE:
        _PROGRAM_CACHE[zero_bias] = build_program(zero_bias=zero_bias)
    return _PROGRAM_CACHE[zero_bias]


def _unpack_core(outT3: np.ndarray) -> np.ndarray:
    # outT3 [P, 2, UM//2, BS]: tile um=2i+e lives at [:, e, i, :]
    tiles = outT3.transpose(2, 1, 0, 3).reshape(UNITS, BS)  # um = 2i+e order
    return tiles.T


def _unpack_outputs(results) -> np.ndarray:
    shards = [_unpack_core(r["outT3"]) for r in results]
    return np.concatenate(shards, axis=0).astype(np.float32)


def kernel(inputs, context, U, S, V, W, bias, _trace=False):
    bias = np.asarray(bias)
    in_maps = _pack_inputs(
        np.asarray(inputs), np.asarray(context), np.asarray(U),
        np.asarray(S), np.asarray(V), np.asarray(W), bias,
    )
    nc = _get_program(zero_bias=not bias.any())
    res = run_bass_kernel_spmd(nc, in_maps, core_ids=list(range(M)), trace=_trace)
    out = _unpack_outputs(res.results)
    if _trace:
        return out, res
    return out
